# revision 3
# baseline (speedup 1.0000x reference)
"""Trainium2 Bass kernel for CGAE graph deconvolution (nn_CGAE_18528488915637).

Reference computation (fp32):
    z_ori  = fadj @ (feat   @ W1)   [N, 64]
    z_aug  = fadj @ (feat_a @ W1)   [N, 64]
    xhat_ori = fadj @ (z_ori @ W2)  [N, 128]
    xhat_aug = fadj @ (z_aug @ W2)  [N, 128]

Sharding: 1D node partition over 8 cores. Core c owns output rows
Ic = [c*2048, (c+1)*2048). Each core streams its fadj row-block (held
transposed so the contraction dim lands on SBUF partitions) through the
PE while the small dense factors (X@W) stay SBUF-resident. The z
all-gather between the two layers happens on host between two launches.

Matmuls run in float32r (fp32 rounded to 11-bit mantissa, full exponent)
at 1 cycle/row — the kernel stays HBM-bandwidth-bound. fp32 accumulate
in PSUM. adj / X@W factors are pre-rounded; measured end-to-end error
vs the fp32 reference is ~2e-4 relative.
"""

import numpy as np

import concourse.bass as bass
import concourse.mybir as mybir
import concourse.tile as tile
from concourse import bacc
from concourse.bass_utils import run_bass_kernel_spmd

N = 16384
NFEAT = 128
NHID = 64
NOUT = 128
NCORES = 8
BLK = N // NCORES  # 2048 rows per core
KT = N // 128      # 128 contraction tiles
IC = BLK // 512    # 4 i-chunks of 512 per core

f32 = mybir.dt.float32
f32r = mybir.dt.float32r

_cache = {}


def round_fp32r(x: np.ndarray) -> np.ndarray:
    """Round fp32 to fp32r encoding (11-bit mantissa, RNE, low 12 bits 0)."""
    b = np.ascontiguousarray(x).view(np.uint32).astype(np.uint64)
    lsb = (b >> 12) & 1
    r = (b + 0x7FF + lsb) & 0xFFFFF000
    return r.astype(np.uint32).view(np.float32)


def build_layer1() -> bass.Bass:
    """Per core: z_t[:, i] = ([feat|feat_a] @ W1).T @ adjT_block[:, i]."""
    nc = bacc.Bacc(None, target_bir_lowering=False)
    adjt = nc.declare_dram_parameter("adjt", [N, BLK], f32r, isOutput=False)
    xt = nc.declare_dram_parameter("xt", [NFEAT, 2 * N], f32, isOutput=False)
    w1 = nc.declare_dram_parameter("w1", [NFEAT, NHID], f32, isOutput=False)
    z_t = nc.declare_dram_parameter("z_t", [2 * NHID, BLK], f32, isOutput=True)

    with tile.TileContext(nc) as tc:
        with (
            tc.tile_pool(name="w", bufs=1) as wp,
            tc.tile_pool(name="s1", bufs=1) as s1p,
            tc.tile_pool(name="xc", bufs=4) as xcp,
            tc.tile_pool(name="adj", bufs=3) as adjp,
            tc.tile_pool(name="out", bufs=2) as outp,
            tc.tile_pool(name="psb", bufs=2, space="PSUM") as psb,
            tc.tile_pool(name="psa", bufs=1, space="PSUM") as psa,
        ):
            w1_sb = wp.tile([NFEAT, NHID], f32, tag="w1")
            nc.sync.dma_start(w1_sb[:], w1[:, :])

            # S1 = [feat @ W1 | feat_a @ W1] in [j, c] layout, fp32r
            s1_sb = s1p.tile([128, KT, 2 * NHID], f32r, tag="s1")
            for jt in range(KT):
                ps = psb.tile([128, 2 * NHID], f32, tag="ps1")
                xo = xcp.tile([NFEAT, 128], f32, tag="xo")
                xa = xcp.tile([NFEAT, 128], f32, tag="xa")
                nc.sync.dma_start(xo[:], xt[:, jt * 128 : (jt + 1) * 128])
                nc.sync.dma_start(xa[:], xt[:, N + jt * 128 : N + (jt + 1) * 128])
                nc.tensor.matmul(ps[:, 0:NHID], xo[:], w1_sb[:], start=True, stop=True)
                nc.tensor.matmul(
                    ps[:, NHID : 2 * NHID], xa[:], w1_sb[:], start=True, stop=True
                )
                nc.vector.tensor_copy(out=s1_sb[:, jt, :], in_=ps[:])

            # z_t[c, i] accumulation over all K tiles, streaming adjT block
            psz = [psa.tile([2 * NHID, 512], f32, tag=f"psz{ic}", name=f"psz{ic}") for ic in range(IC)]
            for jt in range(KT):
                at = adjp.tile([128, BLK], f32r, tag="adj")
                nc.sync.dma_start(at[:], adjt[jt * 128 : (jt + 1) * 128, :])
                for ic in range(IC):
                    nc.tensor.matmul(
                        psz[ic][:],
                        s1_sb[:, jt, :],
                        at[:, ic * 512 : (ic + 1) * 512],
                        start=(jt == 0),
                        stop=(jt == KT - 1),
                    )
            for ic in range(IC):
                zo = outp.tile([2 * NHID, 512], f32, tag="zo")
                nc.vector.tensor_copy(out=zo[:], in_=psz[ic][:])
                nc.sync.dma_start(z_t[:, ic * 512 : (ic + 1) * 512], zo[:])
    nc.compile()
    return nc


def build_layer2() -> bass.Bass:
    """Per core: xhat_t[:, i] = ([z_ori|z_aug] @ W2).T @ adjT_block[:, i]."""
    nc = bacc.Bacc(None, target_bir_lowering=False)
    adjt = nc.declare_dram_parameter("adjt", [N, BLK], f32r, isOutput=False)
    zt = nc.declare_dram_parameter("zt", [2 * NHID, N], f32, isOutput=False)
    w2 = nc.declare_dram_parameter("w2", [NHID, NOUT], f32, isOutput=False)
    xhat_t = nc.declare_dram_parameter("xhat_t", [2 * NOUT, BLK], f32, isOutput=True)

    with tile.TileContext(nc) as tc:
        with (
            tc.tile_pool(name="w", bufs=1) as wp,
            tc.tile_pool(name="s2", bufs=1) as s2p,
            tc.tile_pool(name="zc", bufs=4) as zcp,
            tc.tile_pool(name="adj", bufs=3) as adjp,
            tc.tile_pool(name="out", bufs=2) as outp,
            tc.tile_pool(name="psb", bufs=2, space="PSUM") as psb,
            tc.tile_pool(name="psa", bufs=1, space="PSUM") as psa,
        ):
            w2_sb = wp.tile([NHID, NOUT], f32, tag="w2")
            nc.sync.dma_start(w2_sb[:], w2[:, :])

            # S2 = [z_ori @ W2 | z_aug @ W2] in [j, c2] layout, fp32r
            s2_sb = s2p.tile([128, KT, 2 * NOUT], f32r, tag="s2")
            for jt in range(KT):
                ps = psb.tile([128, 2 * NOUT], f32, tag="ps2")
                zo = zcp.tile([NHID, 128], f32, tag="zco")
                za = zcp.tile([NHID, 128], f32, tag="zca")
                nc.sync.dma_start(zo[:], zt[0:NHID, jt * 128 : (jt + 1) * 128])
                nc.sync.dma_start(
                    za[:], zt[NHID : 2 * NHID, jt * 128 : (jt + 1) * 128]
                )
                nc.tensor.matmul(ps[:, 0:NOUT], zo[:], w2_sb[:], start=True, stop=True)
                nc.tensor.matmul(
                    ps[:, NOUT : 2 * NOUT], za[:], w2_sb[:], start=True, stop=True
                )
                nc.vector.tensor_copy(out=s2_sb[:, jt, :], in_=ps[:])

            # xhat accumulation: 2 passes over i-halves, 4 psum banks each
            for ih in range(2):
                psx = [
                    psa.tile([128, 512], f32, tag=f"psx{q}", name=f"psx{ih}_{q}")
                    for q in range(4)
                ]
                for jt in range(KT):
                    at = adjp.tile([128, BLK // 2], f32r, tag="adj")
                    nc.sync.dma_start(
                        at[:],
                        adjt[
                            jt * 128 : (jt + 1) * 128,
                            ih * (BLK // 2) : (ih + 1) * (BLK // 2),
                        ],
                    )
                    for ch in range(2):  # c2 half: ori / aug
                        for ic in range(2):  # i-chunk within half
                            nc.tensor.matmul(
                                psx[ch * 2 + ic][:],
                                s2_sb[:, jt, ch * NOUT : (ch + 1) * NOUT],
                                at[:, ic * 512 : (ic + 1) * 512],
                                start=(jt == 0),
                                stop=(jt == KT - 1),
                            )
                for ch in range(2):
                    for ic in range(2):
                        xo = outp.tile([128, 512], f32, tag="xo")
                        nc.vector.tensor_copy(out=xo[:], in_=psx[ch * 2 + ic][:])
                        nc.sync.dma_start(
                            xhat_t[
                                ch * NOUT : (ch + 1) * NOUT,
                                ih * (BLK // 2)
                                + ic * 512 : ih * (BLK // 2)
                                + (ic + 1) * 512,
                            ],
                            xo[:],
                        )
    nc.compile()
    return nc


def _prep(feat, feat_a, fadj, W1, W2):
    feat = np.ascontiguousarray(feat, dtype=np.float32)
    feat_a = np.ascontiguousarray(feat_a, dtype=np.float32)
    fadj = np.ascontiguousarray(fadj, dtype=np.float32)
    W1 = np.ascontiguousarray(W1, dtype=np.float32)
    W2 = np.ascontiguousarray(W2, dtype=np.float32)

    adjt = round_fp32r(fadj.T)  # [N, N] (j, i), fp32r-rounded
    adj_blocks = [
        np.ascontiguousarray(adjt[:, c * BLK : (c + 1) * BLK]) for c in range(NCORES)
    ]
    xt = np.ascontiguousarray(np.concatenate([feat, feat_a], axis=0).T)  # [128, 2N]
    return adj_blocks, xt, W1, W2


def _run(inputs, trace=False):
    adj_blocks, xt, W1, W2 = _prep(
        inputs["feat"], inputs["feat_a"], inputs["fadj"], inputs["W1"], inputs["W2"]
    )
    core_ids = list(range(NCORES))

    if "l1" not in _cache:
        _cache["l1"] = build_layer1()
    in_maps1 = [
        {"adjt": adj_blocks[c], "xt": xt, "w1": W1} for c in range(NCORES)
    ]
    r1 = run_bass_kernel_spmd(_cache["l1"], in_maps1, core_ids, trace=trace)
    zt_full = np.concatenate([r1.results[c]["z_t"] for c in range(NCORES)], axis=1)

    if "l2" not in _cache:
        _cache["l2"] = build_layer2()
    in_maps2 = [
        {"adjt": adj_blocks[c], "zt": zt_full, "w2": W2} for c in range(NCORES)
    ]
    r2 = run_bass_kernel_spmd(_cache["l2"], in_maps2, core_ids, trace=trace)
    xhatt = np.concatenate([r2.results[c]["xhat_t"] for c in range(NCORES)], axis=1)

    z_ori = np.ascontiguousarray(zt_full[0:NHID].T)
    z_aug = np.ascontiguousarray(zt_full[NHID : 2 * NHID].T)
    xhat_ori = np.ascontiguousarray(xhatt[0:NOUT].T)
    xhat_aug = np.ascontiguousarray(xhatt[NOUT : 2 * NOUT].T)
    times = (r1.exec_time_ns, r2.exec_time_ns)
    return (z_ori, z_aug, xhat_ori, xhat_aug), times


def kernel(**inputs):
    outputs, _ = _run(inputs, trace=False)
    return outputs


# revision 4
# speedup vs baseline: 1.6675x; 1.6675x over previous
"""Trainium2 Bass kernel for CGAE graph deconvolution (nn_CGAE_18528488915637).

Reference computation (fp32):
    z_ori  = fadj @ (feat   @ W1)   [N, 64]
    z_aug  = fadj @ (feat_a @ W1)   [N, 64]
    xhat_ori = fadj @ (z_ori @ W2)  [N, 128]
    xhat_aug = fadj @ (z_aug @ W2)  [N, 128]

Sharding: 1D node partition over 8 cores; core c owns output rows
Ic = [c*2048, (c+1)*2048). Each core streams its fadj block (transposed
on host so the contraction dim lands on SBUF partitions) through the PE
once per layer — the kernel is HBM-bandwidth-bound on that stream.

Both layers use the associativity adj @ (X @ W) = (adj @ X) @ W: the
node-feature matrix X is already in [contraction, free] layout, so it
feeds the PE stationary side directly with no transposes or
intermediate factor builds; the tiny @W stage runs on the [2048, *]
per-core result in fp32.

Big matmuls run in float32r (fp32 rounded to 11-bit mantissa, full
exponent) at 1 cycle/row with fp32 PSUM accumulation; measured
end-to-end error vs the fp32 reference is ~2e-4 relative.
"""

import numpy as np

import concourse.bass as bass
import concourse.mybir as mybir
import concourse.tile as tile
from concourse import bacc
from concourse.bass_utils import run_bass_kernel_spmd

N = 16384
NFEAT = 128
NHID = 64
NOUT = 128
NCORES = 8
BLK = N // NCORES  # 2048 rows per core
KT = N // 128      # 128 contraction tiles
IC = BLK // 512    # 4 i-chunks of 512 per core

f32 = mybir.dt.float32
f32r = mybir.dt.float32r

_cache = {}


def round_fp32r(x: np.ndarray) -> np.ndarray:
    """Round fp32 to fp32r encoding (11-bit mantissa, RNE, low 12 bits 0)."""
    b = np.ascontiguousarray(x, dtype=np.float32).view(np.uint32).astype(np.uint64)
    lsb = (b >> 12) & 1
    r = (b + 0x7FF + lsb) & 0xFFFFF000
    return r.astype(np.uint32).view(np.float32)


def build_layer1() -> bass.Bass:
    """Per core: Y = adj_blk @ [feat|feat_a] (streamed), then z = Y @ W1."""
    nc = bacc.Bacc(None, target_bir_lowering=False)
    adjt = nc.declare_dram_parameter("adjt", [N, BLK], f32r, isOutput=False)
    xo_d = nc.declare_dram_parameter("xo", [N, NFEAT], f32r, isOutput=False)
    xa_d = nc.declare_dram_parameter("xa", [N, NFEAT], f32r, isOutput=False)
    w1 = nc.declare_dram_parameter("w1", [NFEAT, NHID], f32, isOutput=False)
    z_t = nc.declare_dram_parameter("z_t", [2 * NHID, BLK], f32, isOutput=True)

    with tile.TileContext(nc) as tc:
        with (
            tc.tile_pool(name="w", bufs=1) as wp,
            tc.tile_pool(name="xc", bufs=4) as xcp,
            tc.tile_pool(name="adj", bufs=4) as adjp,
            tc.tile_pool(name="ysb", bufs=2) as ysbp,
            tc.tile_pool(name="out", bufs=2) as outp,
            tc.tile_pool(name="psum", bufs=1, space="PSUM") as psp,
        ):
            w1_sb = wp.tile([NFEAT, NHID], f32, tag="w1")
            nc.sync.dma_start(w1_sb[:], w1[:, :])

            psY = [
                psp.tile([128, 512], f32, tag=f"psY{s}_{ic}", name=f"psY{s}_{ic}")
                for s in range(2)
                for ic in range(IC)
            ]
            for jt in range(KT):
                xo = xcp.tile([128, NFEAT], f32r, tag="xo")
                xa = xcp.tile([128, NFEAT], f32r, tag="xa")
                at = adjp.tile([128, BLK], f32r, tag="adj")
                nc.sync.dma_start(xo[:], xo_d[jt * 128 : (jt + 1) * 128, :])
                nc.sync.dma_start(xa[:], xa_d[jt * 128 : (jt + 1) * 128, :])
                nc.sync.dma_start(at[:], adjt[jt * 128 : (jt + 1) * 128, :])
                for s, xs in enumerate((xo, xa)):
                    for ic in range(IC):
                        nc.tensor.matmul(
                            psY[s * IC + ic][:],
                            xs[:],
                            at[:, ic * 512 : (ic + 1) * 512],
                            start=(jt == 0),
                            stop=(jt == KT - 1),
                        )

            # z.T chunk = W1.T @ Y.T chunk (fp32, tiny)
            for ic in range(IC):
                yo_sb = ysbp.tile([128, 512], f32, tag="yo")
                ya_sb = ysbp.tile([128, 512], f32, tag="ya")
                nc.vector.tensor_copy(out=yo_sb[:], in_=psY[ic][:])
                nc.vector.tensor_copy(out=ya_sb[:], in_=psY[IC + ic][:])
                psz = psp.tile([128, 512], f32, tag=f"psY0_{ic}", name=f"psz{ic}")
                nc.tensor.matmul(
                    psz[0:NHID, :], w1_sb[:], yo_sb[:], start=True, stop=True
                )
                nc.tensor.matmul(
                    psz[NHID : 2 * NHID, :], w1_sb[:], ya_sb[:], start=True, stop=True
                )
                zo = outp.tile([2 * NHID, 512], f32, tag="zo")
                nc.vector.tensor_copy(out=zo[:], in_=psz[:])
                nc.sync.dma_start(z_t[:, ic * 512 : (ic + 1) * 512], zo[:])
    nc.compile()
    return nc


def build_layer2() -> bass.Bass:
    """Per core: U = adj_blk @ [z_ori|z_aug] (streamed), then xhat = U @ W2."""
    nc = bacc.Bacc(None, target_bir_lowering=False)
    adjt = nc.declare_dram_parameter("adjt", [N, BLK], f32r, isOutput=False)
    z_d = nc.declare_dram_parameter("z", [N, 2 * NHID], f32r, isOutput=False)
    w2 = nc.declare_dram_parameter("w2", [NHID, NOUT], f32, isOutput=False)
    xhat_t = nc.declare_dram_parameter("xhat_t", [2 * NOUT, BLK], f32, isOutput=True)

    with tile.TileContext(nc) as tc:
        with (
            tc.tile_pool(name="w", bufs=1) as wp,
            tc.tile_pool(name="zc", bufs=4) as zcp,
            tc.tile_pool(name="adj", bufs=4) as adjp,
            tc.tile_pool(name="usb", bufs=2) as usbp,
            tc.tile_pool(name="out", bufs=2) as outp,
            tc.tile_pool(name="psum", bufs=1, space="PSUM") as psp,
        ):
            # W2 duplicated on both partition halves so each stream's
            # stage-2 matmul finds lhsT/rhs on matching base partitions.
            w2_sb = wp.tile([128, NOUT], f32, tag="w2")
            nc.sync.dma_start(w2_sb[0:NHID, :], w2[:, :])
            nc.sync.dma_start(w2_sb[NHID : 2 * NHID, :], w2[:, :])

            psU = [
                psp.tile([128, 512], f32, tag=f"psU{ic}", name=f"psU{ic}")
                for ic in range(IC)
            ]
            for jt in range(KT):
                zc = zcp.tile([128, 2 * NHID], f32r, tag="zc")
                at = adjp.tile([128, BLK], f32r, tag="adj")
                nc.sync.dma_start(zc[:], z_d[jt * 128 : (jt + 1) * 128, :])
                nc.sync.dma_start(at[:], adjt[jt * 128 : (jt + 1) * 128, :])
                for ic in range(IC):
                    nc.tensor.matmul(
                        psU[ic][:],
                        zc[:],
                        at[:, ic * 512 : (ic + 1) * 512],
                        start=(jt == 0),
                        stop=(jt == KT - 1),
                    )

            # xhat.T chunk = W2.T @ U.T chunk per stream (fp32, tiny)
            for ic in range(IC):
                u_sb = usbp.tile([128, 512], f32, tag="u")
                nc.vector.tensor_copy(out=u_sb[:], in_=psU[ic][:])
                for s in range(2):
                    psx = psp.tile(
                        [128, 512],
                        f32,
                        tag=f"psU{ic}" if s == 0 else "psx1",
                        name=f"psx{ic}_{s}",
                    )
                    nc.tensor.matmul(
                        psx[:],
                        w2_sb[s * NHID : (s + 1) * NHID, :],
                        u_sb[s * NHID : (s + 1) * NHID, :],
                        start=True,
                        stop=True,
                    )
                    xo = outp.tile([NOUT, 512], f32, tag="xho")
                    nc.vector.tensor_copy(out=xo[:], in_=psx[:NOUT, :])
                    nc.sync.dma_start(
                        xhat_t[s * NOUT : (s + 1) * NOUT, ic * 512 : (ic + 1) * 512],
                        xo[:],
                    )
    nc.compile()
    return nc


def _prep(feat, feat_a, fadj, W1, W2):
    feat = np.ascontiguousarray(feat, dtype=np.float32)
    feat_a = np.ascontiguousarray(feat_a, dtype=np.float32)
    fadj = np.ascontiguousarray(fadj, dtype=np.float32)
    W1 = np.ascontiguousarray(W1, dtype=np.float32)
    W2 = np.ascontiguousarray(W2, dtype=np.float32)

    adjt = round_fp32r(fadj.T)  # [N, N] = (j, i), fp32r-rounded
    adj_blocks = [
        np.ascontiguousarray(adjt[:, c * BLK : (c + 1) * BLK]) for c in range(NCORES)
    ]
    return adj_blocks, round_fp32r(feat), round_fp32r(feat_a), W1, W2


def _run(inputs, trace=False):
    adj_blocks, xo, xa, W1, W2 = _prep(
        inputs["feat"], inputs["feat_a"], inputs["fadj"], inputs["W1"], inputs["W2"]
    )
    core_ids = list(range(NCORES))

    if "l1" not in _cache:
        _cache["l1"] = build_layer1()
    in_maps1 = [
        {"adjt": adj_blocks[c], "xo": xo, "xa": xa, "w1": W1} for c in range(NCORES)
    ]
    r1 = run_bass_kernel_spmd(_cache["l1"], in_maps1, core_ids, trace=trace)
    zt_full = np.concatenate([r1.results[c]["z_t"] for c in range(NCORES)], axis=1)

    z_ori = np.ascontiguousarray(zt_full[0:NHID].T)
    z_aug = np.ascontiguousarray(zt_full[NHID : 2 * NHID].T)
    z_nat = round_fp32r(np.concatenate([z_ori, z_aug], axis=1))  # [N, 128]

    if "l2" not in _cache:
        _cache["l2"] = build_layer2()
    in_maps2 = [
        {"adjt": adj_blocks[c], "z": z_nat, "w2": W2} for c in range(NCORES)
    ]
    r2 = run_bass_kernel_spmd(_cache["l2"], in_maps2, core_ids, trace=trace)
    xhatt = np.concatenate([r2.results[c]["xhat_t"] for c in range(NCORES)], axis=1)

    xhat_ori = np.ascontiguousarray(xhatt[0:NOUT].T)
    xhat_aug = np.ascontiguousarray(xhatt[NOUT : 2 * NOUT].T)
    times = (r1.exec_time_ns, r2.exec_time_ns)
    return (z_ori, z_aug, xhat_ori, xhat_aug), times


def kernel(**inputs):
    outputs, _ = _run(inputs, trace=False)
    return outputs


# revision 5
# speedup vs baseline: 2.6241x; 1.5737x over previous
"""Trainium2 Bass kernel for CGAE graph deconvolution (nn_CGAE_18528488915637).

Reference computation (fp32):
    z_ori  = fadj @ (feat   @ W1)   [N, 64]
    z_aug  = fadj @ (feat_a @ W1)   [N, 64]
    xhat_ori = fadj @ (z_ori @ W2)  [N, 128]
    xhat_aug = fadj @ (z_aug @ W2)  [N, 128]

Sharding: 1D node partition over 8 cores; core c owns output rows
Ic = [c*2048, (c+1)*2048). Each core streams its fadj block (transposed
on host so the contraction dim lands on SBUF partitions) through the PE
once per layer — the kernel is HBM-bandwidth-bound on that stream.

Both layers use the associativity adj @ (X @ W) = (adj @ X) @ W: the
node-feature matrix X is already in [contraction, free] layout, so it
feeds the PE stationary side directly with no transposes or
intermediate factor builds; the tiny @W stage runs on the [2048, *]
per-core result in fp32.

Big matmuls run in float32r (fp32 rounded to 11-bit mantissa, full
exponent) at 1 cycle/row with fp32 PSUM accumulation; measured
end-to-end error vs the fp32 reference is ~2e-4 relative.
"""

import numpy as np

import concourse.bass as bass
import concourse.mybir as mybir
import concourse.tile as tile
from concourse import bacc
from concourse.bass_utils import run_bass_kernel_spmd

N = 16384
NFEAT = 128
NHID = 64
NOUT = 128
NCORES = 8
BLK = N // NCORES  # 2048 rows per core
KT = N // 128      # 128 contraction tiles
IC = BLK // 512    # 4 i-chunks of 512 per core

f32 = mybir.dt.float32
f32r = mybir.dt.float32r
f16 = mybir.dt.float16

# dtype of the streamed adjacency / node-feature matmuls. "f16" halves
# HBM traffic (measured ~2.5e-4 per-matmul error at K=16384 vs 1.3e-4
# for f32r); "f32r" is the full-bandwidth fp32-range fallback.
BIG = "f16"

_cache = {}


def big_dt():
    return f16 if BIG == "f16" else f32r


def big_cast(x: np.ndarray) -> np.ndarray:
    if BIG == "f16":
        return x.astype(np.float16)
    return round_fp32r(x)


def round_fp32r(x: np.ndarray) -> np.ndarray:
    """Round fp32 to fp32r encoding (11-bit mantissa, RNE, low 12 bits 0)."""
    b = np.ascontiguousarray(x, dtype=np.float32).view(np.uint32).astype(np.uint64)
    lsb = (b >> 12) & 1
    r = (b + 0x7FF + lsb) & 0xFFFFF000
    return r.astype(np.uint32).view(np.float32)


def build_layer1(bdt) -> bass.Bass:
    """Per core: Y = adj_blk @ [feat|feat_a] (streamed), then z = Y @ W1."""
    nc = bacc.Bacc(None, target_bir_lowering=False)
    adjt = nc.declare_dram_parameter("adjt", [N, BLK], bdt, isOutput=False)
    xo_d = nc.declare_dram_parameter("xo", [N, NFEAT], bdt, isOutput=False)
    xa_d = nc.declare_dram_parameter("xa", [N, NFEAT], bdt, isOutput=False)
    w1 = nc.declare_dram_parameter("w1", [NFEAT, NHID], f32, isOutput=False)
    z_t = nc.declare_dram_parameter("z_t", [2 * NHID, BLK], f32, isOutput=True)

    with tile.TileContext(nc) as tc:
        with (
            tc.tile_pool(name="w", bufs=1) as wp,
            tc.tile_pool(name="xc", bufs=4) as xcp,
            tc.tile_pool(name="adj", bufs=4) as adjp,
            tc.tile_pool(name="ysb", bufs=2) as ysbp,
            tc.tile_pool(name="out", bufs=2) as outp,
            tc.tile_pool(name="psum", bufs=1, space="PSUM") as psp,
        ):
            w1_sb = wp.tile([NFEAT, NHID], f32, tag="w1")
            nc.sync.dma_start(w1_sb[:], w1[:, :])

            psY = [
                psp.tile([128, 512], f32, tag=f"psY{s}_{ic}", name=f"psY{s}_{ic}")
                for s in range(2)
                for ic in range(IC)
            ]
            for jt in range(KT):
                xo = xcp.tile([128, NFEAT], bdt, tag="xo")
                xa = xcp.tile([128, NFEAT], bdt, tag="xa")
                at = adjp.tile([128, BLK], bdt, tag="adj")
                nc.sync.dma_start(xo[:], xo_d[jt * 128 : (jt + 1) * 128, :])
                nc.sync.dma_start(xa[:], xa_d[jt * 128 : (jt + 1) * 128, :])
                nc.sync.dma_start(at[:], adjt[jt * 128 : (jt + 1) * 128, :])
                for s, xs in enumerate((xo, xa)):
                    for ic in range(IC):
                        nc.tensor.matmul(
                            psY[s * IC + ic][:],
                            xs[:],
                            at[:, ic * 512 : (ic + 1) * 512],
                            start=(jt == 0),
                            stop=(jt == KT - 1),
                        )

            # z.T chunk = W1.T @ Y.T chunk (fp32, tiny)
            for ic in range(IC):
                yo_sb = ysbp.tile([128, 512], f32, tag="yo")
                ya_sb = ysbp.tile([128, 512], f32, tag="ya")
                nc.vector.tensor_copy(out=yo_sb[:], in_=psY[ic][:])
                nc.vector.tensor_copy(out=ya_sb[:], in_=psY[IC + ic][:])
                psz = psp.tile([128, 512], f32, tag=f"psY0_{ic}", name=f"psz{ic}")
                nc.tensor.matmul(
                    psz[0:NHID, :], w1_sb[:], yo_sb[:], start=True, stop=True
                )
                nc.tensor.matmul(
                    psz[NHID : 2 * NHID, :], w1_sb[:], ya_sb[:], start=True, stop=True
                )
                zo = outp.tile([2 * NHID, 512], f32, tag="zo")
                nc.vector.tensor_copy(out=zo[:], in_=psz[:])
                nc.sync.dma_start(z_t[:, ic * 512 : (ic + 1) * 512], zo[:])
    nc.compile()
    return nc


def build_layer2(bdt) -> bass.Bass:
    """Per core: U = adj_blk @ [z_ori|z_aug] (streamed), then xhat = U @ W2."""
    nc = bacc.Bacc(None, target_bir_lowering=False)
    adjt = nc.declare_dram_parameter("adjt", [N, BLK], bdt, isOutput=False)
    z_d = nc.declare_dram_parameter("z", [N, 2 * NHID], bdt, isOutput=False)
    w2 = nc.declare_dram_parameter("w2", [NHID, NOUT], f32, isOutput=False)
    xhat_t = nc.declare_dram_parameter("xhat_t", [2 * NOUT, BLK], f32, isOutput=True)

    with tile.TileContext(nc) as tc:
        with (
            tc.tile_pool(name="w", bufs=1) as wp,
            tc.tile_pool(name="zc", bufs=4) as zcp,
            tc.tile_pool(name="adj", bufs=4) as adjp,
            tc.tile_pool(name="usb", bufs=2) as usbp,
            tc.tile_pool(name="out", bufs=2) as outp,
            tc.tile_pool(name="psum", bufs=1, space="PSUM") as psp,
        ):
            # W2 duplicated on both partition halves so each stream's
            # stage-2 matmul finds lhsT/rhs on matching base partitions.
            w2_sb = wp.tile([128, NOUT], f32, tag="w2")
            nc.sync.dma_start(w2_sb[0:NHID, :], w2[:, :])
            nc.sync.dma_start(w2_sb[NHID : 2 * NHID, :], w2[:, :])

            psU = [
                psp.tile([128, 512], f32, tag=f"psU{ic}", name=f"psU{ic}")
                for ic in range(IC)
            ]
            for jt in range(KT):
                zc = zcp.tile([128, 2 * NHID], bdt, tag="zc")
                at = adjp.tile([128, BLK], bdt, tag="adj")
                nc.sync.dma_start(zc[:], z_d[jt * 128 : (jt + 1) * 128, :])
                nc.sync.dma_start(at[:], adjt[jt * 128 : (jt + 1) * 128, :])
                for ic in range(IC):
                    nc.tensor.matmul(
                        psU[ic][:],
                        zc[:],
                        at[:, ic * 512 : (ic + 1) * 512],
                        start=(jt == 0),
                        stop=(jt == KT - 1),
                    )

            # xhat.T chunk = W2.T @ U.T chunk per stream (fp32, tiny)
            for ic in range(IC):
                u_sb = usbp.tile([128, 512], f32, tag="u")
                nc.vector.tensor_copy(out=u_sb[:], in_=psU[ic][:])
                for s in range(2):
                    psx = psp.tile(
                        [128, 512],
                        f32,
                        tag=f"psU{ic}" if s == 0 else "psx1",
                        name=f"psx{ic}_{s}",
                    )
                    nc.tensor.matmul(
                        psx[:],
                        w2_sb[s * NHID : (s + 1) * NHID, :],
                        u_sb[s * NHID : (s + 1) * NHID, :],
                        start=True,
                        stop=True,
                    )
                    xo = outp.tile([NOUT, 512], f32, tag="xho")
                    nc.vector.tensor_copy(out=xo[:], in_=psx[:NOUT, :])
                    nc.sync.dma_start(
                        xhat_t[s * NOUT : (s + 1) * NOUT, ic * 512 : (ic + 1) * 512],
                        xo[:],
                    )
    nc.compile()
    return nc


def _prep(feat, feat_a, fadj, W1, W2):
    feat = np.ascontiguousarray(feat, dtype=np.float32)
    feat_a = np.ascontiguousarray(feat_a, dtype=np.float32)
    fadj = np.ascontiguousarray(fadj, dtype=np.float32)
    W1 = np.ascontiguousarray(W1, dtype=np.float32)
    W2 = np.ascontiguousarray(W2, dtype=np.float32)

    adjt = big_cast(fadj.T)  # [N, N] = (j, i)
    adj_blocks = [
        np.ascontiguousarray(adjt[:, c * BLK : (c + 1) * BLK]) for c in range(NCORES)
    ]
    return adj_blocks, big_cast(feat), big_cast(feat_a), W1, W2


def _run(inputs, trace=False):
    adj_blocks, xo, xa, W1, W2 = _prep(
        inputs["feat"], inputs["feat_a"], inputs["fadj"], inputs["W1"], inputs["W2"]
    )
    core_ids = list(range(NCORES))

    if ("l1", BIG) not in _cache:
        _cache[("l1", BIG)] = build_layer1(big_dt())
    in_maps1 = [
        {"adjt": adj_blocks[c], "xo": xo, "xa": xa, "w1": W1} for c in range(NCORES)
    ]
    r1 = run_bass_kernel_spmd(_cache[("l1", BIG)], in_maps1, core_ids, trace=trace)
    zt_full = np.concatenate([r1.results[c]["z_t"] for c in range(NCORES)], axis=1)

    z_ori = np.ascontiguousarray(zt_full[0:NHID].T)
    z_aug = np.ascontiguousarray(zt_full[NHID : 2 * NHID].T)
    z_nat = big_cast(np.concatenate([z_ori, z_aug], axis=1))  # [N, 128]

    if ("l2", BIG) not in _cache:
        _cache[("l2", BIG)] = build_layer2(big_dt())
    in_maps2 = [
        {"adjt": adj_blocks[c], "z": z_nat, "w2": W2} for c in range(NCORES)
    ]
    r2 = run_bass_kernel_spmd(_cache[("l2", BIG)], in_maps2, core_ids, trace=trace)
    xhatt = np.concatenate([r2.results[c]["xhat_t"] for c in range(NCORES)], axis=1)

    xhat_ori = np.ascontiguousarray(xhatt[0:NOUT].T)
    xhat_aug = np.ascontiguousarray(xhatt[NOUT : 2 * NOUT].T)
    times = (r1.exec_time_ns, r2.exec_time_ns)
    return (z_ori, z_aug, xhat_ori, xhat_aug), times


def kernel(**inputs):
    outputs, _ = _run(inputs, trace=False)
    return outputs


# revision 7
# speedup vs baseline: 2.6297x; 1.0021x over previous
"""Trainium2 Bass kernel for CGAE graph deconvolution (nn_CGAE_18528488915637).

Reference computation (fp32):
    z_ori  = fadj @ (feat   @ W1)   [N, 64]
    z_aug  = fadj @ (feat_a @ W1)   [N, 64]
    xhat_ori = fadj @ (z_ori @ W2)  [N, 128]
    xhat_aug = fadj @ (z_aug @ W2)  [N, 128]

Sharding: 1D node partition over 8 cores; core c owns output rows
Ic = [c*2048, (c+1)*2048). Each core streams its fadj block (transposed
on host so the contraction dim lands on SBUF partitions) through the PE
once per layer — the kernel is HBM-bandwidth-bound on that stream.

Both layers use the associativity adj @ (X @ W) = (adj @ X) @ W: the
node-feature matrix X is already in [contraction, free] layout, so it
feeds the PE stationary side directly with no transposes or
intermediate factor builds; the tiny @W stage runs on the [2048, *]
per-core result in fp32.

Big matmuls run in float32r (fp32 rounded to 11-bit mantissa, full
exponent) at 1 cycle/row with fp32 PSUM accumulation; measured
end-to-end error vs the fp32 reference is ~2e-4 relative.
"""

import numpy as np

import concourse.bass as bass
import concourse.mybir as mybir
import concourse.tile as tile
from concourse import bacc
from concourse.bass_utils import run_bass_kernel_spmd

N = 16384
NFEAT = 128
NHID = 64
NOUT = 128
NCORES = 8
BLK = N // NCORES  # 2048 rows per core
KT = N // 128      # 128 contraction tiles
IC = BLK // 512    # 4 i-chunks of 512 per core

f32 = mybir.dt.float32
f32r = mybir.dt.float32r
f16 = mybir.dt.float16

# dtype of the streamed adjacency / node-feature matmuls. "f16" halves
# HBM traffic (measured ~2.5e-4 per-matmul error at K=16384 vs 1.3e-4
# for f32r); "f32r" is the full-bandwidth fp32-range fallback.
BIG = "f16"

_cache = {}


def big_dt():
    return f16 if BIG == "f16" else f32r


def big_cast(x: np.ndarray) -> np.ndarray:
    if BIG == "f16":
        return x.astype(np.float16)
    return round_fp32r(x)


def round_fp32r(x: np.ndarray) -> np.ndarray:
    """Round fp32 to fp32r encoding (11-bit mantissa, RNE, low 12 bits 0)."""
    b = np.ascontiguousarray(x, dtype=np.float32).view(np.uint32).astype(np.uint64)
    lsb = (b >> 12) & 1
    r = (b + 0x7FF + lsb) & 0xFFFFF000
    return r.astype(np.uint32).view(np.float32)


def build_layer1(bdt) -> bass.Bass:
    """Per core: Y = adj_blk @ [feat|feat_a] (streamed), then z = Y @ W1."""
    nc = bacc.Bacc(None, target_bir_lowering=False)
    adjt = nc.declare_dram_parameter("adjt", [N, BLK], bdt, isOutput=False)
    xo_d = nc.declare_dram_parameter("xo", [N, NFEAT], bdt, isOutput=False)
    xa_d = nc.declare_dram_parameter("xa", [N, NFEAT], bdt, isOutput=False)
    w1 = nc.declare_dram_parameter("w1", [NFEAT, NHID], f32, isOutput=False)
    z_t = nc.declare_dram_parameter("z_t", [2 * NHID, BLK], f32, isOutput=True)

    with tile.TileContext(nc) as tc:
        with (
            tc.tile_pool(name="w", bufs=1) as wp,
            tc.tile_pool(name="adj", bufs=4) as adjp,
            tc.tile_pool(name="ysb", bufs=2) as ysbp,
            tc.tile_pool(name="out", bufs=2) as outp,
            tc.tile_pool(name="psum", bufs=1, space="PSUM") as psp,
        ):
            w1_sb = wp.tile([NFEAT, NHID], f32, tag="w1")
            nc.scalar.dma_start(w1_sb[:], w1[:, :])

            # node features resident in SBUF, [j_inner, j_tile, f] layout,
            # loaded in chunks on the scalar DMA ring so the sync ring
            # carries nothing but the adjacency stream.
            xo_sb = wp.tile([128, KT, NFEAT], bdt, tag="xo_sb")
            xa_sb = wp.tile([128, KT, NFEAT], bdt, tag="xa_sb")
            NCH = 8
            CH = KT // NCH
            for c in range(NCH):
                nc.scalar.dma_start(
                    xo_sb[:, c * CH : (c + 1) * CH, :],
                    xo_d[c * CH * 128 : (c + 1) * CH * 128, :].rearrange(
                        "(o p) f -> p o f", p=128
                    ),
                )
                nc.scalar.dma_start(
                    xa_sb[:, c * CH : (c + 1) * CH, :],
                    xa_d[c * CH * 128 : (c + 1) * CH * 128, :].rearrange(
                        "(o p) f -> p o f", p=128
                    ),
                )

            psY = [
                psp.tile([128, 512], f32, tag=f"psY{s}_{ic}", name=f"psY{s}_{ic}")
                for s in range(2)
                for ic in range(IC)
            ]
            for jt in range(KT):
                at = adjp.tile([128, BLK], bdt, tag="adj")
                nc.sync.dma_start(at[:], adjt[jt * 128 : (jt + 1) * 128, :])
                for s, xs in enumerate((xo_sb, xa_sb)):
                    for ic in range(IC):
                        nc.tensor.matmul(
                            psY[s * IC + ic][:],
                            xs[:, jt, :],
                            at[:, ic * 512 : (ic + 1) * 512],
                            start=(jt == 0),
                            stop=(jt == KT - 1),
                        )

            # z.T chunk = W1.T @ Y.T chunk (fp32, tiny)
            for ic in range(IC):
                yo_sb = ysbp.tile([128, 512], f32, tag="yo")
                ya_sb = ysbp.tile([128, 512], f32, tag="ya")
                nc.vector.tensor_copy(out=yo_sb[:], in_=psY[ic][:])
                nc.vector.tensor_copy(out=ya_sb[:], in_=psY[IC + ic][:])
                psz = psp.tile([128, 512], f32, tag=f"psY0_{ic}", name=f"psz{ic}")
                nc.tensor.matmul(
                    psz[0:NHID, :], w1_sb[:], yo_sb[:], start=True, stop=True
                )
                nc.tensor.matmul(
                    psz[NHID : 2 * NHID, :], w1_sb[:], ya_sb[:], start=True, stop=True
                )
                zo = outp.tile([2 * NHID, 512], f32, tag="zo")
                nc.vector.tensor_copy(out=zo[:], in_=psz[:])
                nc.scalar.dma_start(z_t[:, ic * 512 : (ic + 1) * 512], zo[:])
    nc.compile()
    return nc


def build_layer2(bdt) -> bass.Bass:
    """Per core: U = adj_blk @ [z_ori|z_aug] (streamed), then xhat = U @ W2."""
    nc = bacc.Bacc(None, target_bir_lowering=False)
    adjt = nc.declare_dram_parameter("adjt", [N, BLK], bdt, isOutput=False)
    z_d = nc.declare_dram_parameter("z", [N, 2 * NHID], bdt, isOutput=False)
    w2 = nc.declare_dram_parameter("w2", [NHID, NOUT], f32, isOutput=False)
    xhat_t = nc.declare_dram_parameter("xhat_t", [2 * NOUT, BLK], f32, isOutput=True)

    with tile.TileContext(nc) as tc:
        with (
            tc.tile_pool(name="w", bufs=1) as wp,
            tc.tile_pool(name="adj", bufs=4) as adjp,
            tc.tile_pool(name="usb", bufs=2) as usbp,
            tc.tile_pool(name="out", bufs=2) as outp,
            tc.tile_pool(name="psum", bufs=1, space="PSUM") as psp,
        ):
            # W2 duplicated on both partition halves so each stream's
            # stage-2 matmul finds lhsT/rhs on matching base partitions.
            w2_sb = wp.tile([128, NOUT], f32, tag="w2")
            nc.scalar.dma_start(w2_sb[0:NHID, :], w2[:, :])
            nc.scalar.dma_start(w2_sb[NHID : 2 * NHID, :], w2[:, :])

            z_sb = wp.tile([128, KT, 2 * NHID], bdt, tag="z_sb")
            NCH = 8
            CH = KT // NCH
            for c in range(NCH):
                nc.scalar.dma_start(
                    z_sb[:, c * CH : (c + 1) * CH, :],
                    z_d[c * CH * 128 : (c + 1) * CH * 128, :].rearrange(
                        "(o p) h -> p o h", p=128
                    ),
                )

            psU = [
                psp.tile([128, 512], f32, tag=f"psU{ic}", name=f"psU{ic}")
                for ic in range(IC)
            ]
            for jt in range(KT):
                at = adjp.tile([128, BLK], bdt, tag="adj")
                nc.sync.dma_start(at[:], adjt[jt * 128 : (jt + 1) * 128, :])
                for ic in range(IC):
                    nc.tensor.matmul(
                        psU[ic][:],
                        z_sb[:, jt, :],
                        at[:, ic * 512 : (ic + 1) * 512],
                        start=(jt == 0),
                        stop=(jt == KT - 1),
                    )

            # xhat.T chunk = W2.T @ U.T chunk per stream (fp32, tiny)
            for ic in range(IC):
                u_sb = usbp.tile([128, 512], f32, tag="u")
                nc.vector.tensor_copy(out=u_sb[:], in_=psU[ic][:])
                for s in range(2):
                    psx = psp.tile(
                        [128, 512],
                        f32,
                        tag=f"psU{ic}" if s == 0 else "psx1",
                        name=f"psx{ic}_{s}",
                    )
                    nc.tensor.matmul(
                        psx[:],
                        w2_sb[s * NHID : (s + 1) * NHID, :],
                        u_sb[s * NHID : (s + 1) * NHID, :],
                        start=True,
                        stop=True,
                    )
                    xo = outp.tile([NOUT, 512], f32, tag="xho")
                    nc.vector.tensor_copy(out=xo[:], in_=psx[:NOUT, :])
                    nc.scalar.dma_start(
                        xhat_t[s * NOUT : (s + 1) * NOUT, ic * 512 : (ic + 1) * 512],
                        xo[:],
                    )
    nc.compile()
    return nc


def _prep(feat, feat_a, fadj, W1, W2):
    feat = np.ascontiguousarray(feat, dtype=np.float32)
    feat_a = np.ascontiguousarray(feat_a, dtype=np.float32)
    fadj = np.ascontiguousarray(fadj, dtype=np.float32)
    W1 = np.ascontiguousarray(W1, dtype=np.float32)
    W2 = np.ascontiguousarray(W2, dtype=np.float32)

    adjt = big_cast(fadj.T)  # [N, N] = (j, i)
    adj_blocks = [
        np.ascontiguousarray(adjt[:, c * BLK : (c + 1) * BLK]) for c in range(NCORES)
    ]
    return adj_blocks, big_cast(feat), big_cast(feat_a), W1, W2


def _run(inputs, trace=False):
    adj_blocks, xo, xa, W1, W2 = _prep(
        inputs["feat"], inputs["feat_a"], inputs["fadj"], inputs["W1"], inputs["W2"]
    )
    core_ids = list(range(NCORES))

    if ("l1", BIG) not in _cache:
        _cache[("l1", BIG)] = build_layer1(big_dt())
    in_maps1 = [
        {"adjt": adj_blocks[c], "xo": xo, "xa": xa, "w1": W1} for c in range(NCORES)
    ]
    r1 = run_bass_kernel_spmd(_cache[("l1", BIG)], in_maps1, core_ids, trace=trace)
    zt_full = np.concatenate([r1.results[c]["z_t"] for c in range(NCORES)], axis=1)

    z_ori = np.ascontiguousarray(zt_full[0:NHID].T)
    z_aug = np.ascontiguousarray(zt_full[NHID : 2 * NHID].T)
    z_nat = big_cast(np.concatenate([z_ori, z_aug], axis=1))  # [N, 128]

    if ("l2", BIG) not in _cache:
        _cache[("l2", BIG)] = build_layer2(big_dt())
    in_maps2 = [
        {"adjt": adj_blocks[c], "z": z_nat, "w2": W2} for c in range(NCORES)
    ]
    r2 = run_bass_kernel_spmd(_cache[("l2", BIG)], in_maps2, core_ids, trace=trace)
    xhatt = np.concatenate([r2.results[c]["xhat_t"] for c in range(NCORES)], axis=1)

    xhat_ori = np.ascontiguousarray(xhatt[0:NOUT].T)
    xhat_aug = np.ascontiguousarray(xhatt[NOUT : 2 * NOUT].T)
    times = (r1.exec_time_ns, r2.exec_time_ns)
    return (z_ori, z_aug, xhat_ori, xhat_aug), times


def kernel(**inputs):
    outputs, _ = _run(inputs, trace=False)
    return outputs


# revision 8
# speedup vs baseline: 2.8895x; 1.0988x over previous
"""Trainium2 Bass kernel for CGAE graph deconvolution (nn_CGAE_18528488915637).

Reference computation (fp32):
    z_ori  = fadj @ (feat   @ W1)   [N, 64]
    z_aug  = fadj @ (feat_a @ W1)   [N, 64]
    xhat_ori = fadj @ (z_ori @ W2)  [N, 128]
    xhat_aug = fadj @ (z_aug @ W2)  [N, 128]

Sharding: 1D node partition over 8 cores; core c owns output rows
Ic = [c*2048, (c+1)*2048). Each core streams its fadj block (transposed
on host so the contraction dim lands on SBUF partitions) through the PE
once per layer — the kernel is HBM-bandwidth-bound on that stream.

Both layers use the associativity adj @ (X @ W) = (adj @ X) @ W: the
node-feature matrix X is already in [contraction, free] layout, so it
feeds the PE stationary side directly with no transposes or
intermediate factor builds; the tiny @W stage runs on the [2048, *]
per-core result in fp32.

Big matmuls run in float32r (fp32 rounded to 11-bit mantissa, full
exponent) at 1 cycle/row with fp32 PSUM accumulation; measured
end-to-end error vs the fp32 reference is ~2e-4 relative.
"""

import numpy as np

import concourse.bass as bass
import concourse.mybir as mybir
import concourse.tile as tile
from concourse import bacc
from concourse.bass_utils import run_bass_kernel_spmd

N = 16384
NFEAT = 128
NHID = 64
NOUT = 128
NCORES = 8
BLK = N // NCORES  # 2048 rows per core
KT = N // 128      # 128 contraction tiles
IC = BLK // 512    # 4 i-chunks of 512 per core

f32 = mybir.dt.float32
f32r = mybir.dt.float32r
f16 = mybir.dt.float16

# dtype of the streamed adjacency / node-feature matmuls. "f16" halves
# HBM traffic (measured ~2.5e-4 per-matmul error at K=16384 vs 1.3e-4
# for f32r); "f32r" is the full-bandwidth fp32-range fallback.
BIG = "f16"

_cache = {}


def big_dt():
    return f16 if BIG == "f16" else f32r


def big_cast(x: np.ndarray) -> np.ndarray:
    if BIG == "f16":
        return x.astype(np.float16)
    return round_fp32r(x)


def round_fp32r(x: np.ndarray) -> np.ndarray:
    """Round fp32 to fp32r encoding (11-bit mantissa, RNE, low 12 bits 0)."""
    b = np.ascontiguousarray(x, dtype=np.float32).view(np.uint32).astype(np.uint64)
    lsb = (b >> 12) & 1
    r = (b + 0x7FF + lsb) & 0xFFFFF000
    return r.astype(np.uint32).view(np.float32)


def build_layer1(bdt) -> bass.Bass:
    """Per core: Y = adj_blk @ [feat|feat_a] (streamed), then z = Y @ W1."""
    nc = bacc.Bacc(None, target_bir_lowering=False)
    adjt = nc.declare_dram_parameter("adjt", [N, BLK], bdt, isOutput=False)
    xo_d = nc.declare_dram_parameter("xo", [N, NFEAT], bdt, isOutput=False)
    xa_d = nc.declare_dram_parameter("xa", [N, NFEAT], bdt, isOutput=False)
    w1 = nc.declare_dram_parameter("w1", [NFEAT, NHID], f32, isOutput=False)
    z_t = nc.declare_dram_parameter("z_t", [2 * NHID, BLK], f32, isOutput=True)

    with tile.TileContext(nc) as tc:
        with (
            tc.tile_pool(name="w", bufs=1) as wp,
            tc.tile_pool(name="adj", bufs=8) as adjp,
            tc.tile_pool(name="ysb", bufs=2) as ysbp,
            tc.tile_pool(name="out", bufs=2) as outp,
            tc.tile_pool(name="psum", bufs=1, space="PSUM") as psp,
        ):
            w1_sb = wp.tile([NFEAT, NHID], f32, tag="w1")
            nc.scalar.dma_start(w1_sb[:], w1[:, :])

            # node features resident in SBUF, [j_inner, j_tile, f] layout,
            # loaded in chunks on the scalar DMA ring so the sync ring
            # carries nothing but the adjacency stream.
            xo_sb = wp.tile([128, KT, NFEAT], bdt, tag="xo_sb")
            xa_sb = wp.tile([128, KT, NFEAT], bdt, tag="xa_sb")
            NCH = 8
            CH = KT // NCH
            for c in range(NCH):
                nc.scalar.dma_start(
                    xo_sb[:, c * CH : (c + 1) * CH, :],
                    xo_d[c * CH * 128 : (c + 1) * CH * 128, :].rearrange(
                        "(o p) f -> p o f", p=128
                    ),
                )
                nc.scalar.dma_start(
                    xa_sb[:, c * CH : (c + 1) * CH, :],
                    xa_d[c * CH * 128 : (c + 1) * CH * 128, :].rearrange(
                        "(o p) f -> p o f", p=128
                    ),
                )

            psY = [
                psp.tile([128, 512], f32, tag=f"psY{s}_{ic}", name=f"psY{s}_{ic}")
                for s in range(2)
                for ic in range(IC)
            ]
            for jt in range(KT):
                at = adjp.tile([128, BLK], bdt, tag="adj")
                eng = nc.sync if jt % 2 == 0 else nc.scalar
                eng.dma_start(at[:], adjt[jt * 128 : (jt + 1) * 128, :])
                for s, xs in enumerate((xo_sb, xa_sb)):
                    for ic in range(IC):
                        nc.tensor.matmul(
                            psY[s * IC + ic][:],
                            xs[:, jt, :],
                            at[:, ic * 512 : (ic + 1) * 512],
                            start=(jt == 0),
                            stop=(jt == KT - 1),
                        )

            # z.T chunk = W1.T @ Y.T chunk (fp32, tiny)
            for ic in range(IC):
                yo_sb = ysbp.tile([128, 512], f32, tag="yo")
                ya_sb = ysbp.tile([128, 512], f32, tag="ya")
                nc.vector.tensor_copy(out=yo_sb[:], in_=psY[ic][:])
                nc.vector.tensor_copy(out=ya_sb[:], in_=psY[IC + ic][:])
                psz = psp.tile([128, 512], f32, tag=f"psY0_{ic}", name=f"psz{ic}")
                nc.tensor.matmul(
                    psz[0:NHID, :], w1_sb[:], yo_sb[:], start=True, stop=True
                )
                nc.tensor.matmul(
                    psz[NHID : 2 * NHID, :], w1_sb[:], ya_sb[:], start=True, stop=True
                )
                zo = outp.tile([2 * NHID, 512], f32, tag="zo")
                nc.vector.tensor_copy(out=zo[:], in_=psz[:])
                nc.scalar.dma_start(z_t[:, ic * 512 : (ic + 1) * 512], zo[:])
    nc.compile()
    return nc


def build_layer2(bdt) -> bass.Bass:
    """Per core: U = adj_blk @ [z_ori|z_aug] (streamed), then xhat = U @ W2."""
    nc = bacc.Bacc(None, target_bir_lowering=False)
    adjt = nc.declare_dram_parameter("adjt", [N, BLK], bdt, isOutput=False)
    z_d = nc.declare_dram_parameter("z", [N, 2 * NHID], bdt, isOutput=False)
    w2 = nc.declare_dram_parameter("w2", [NHID, NOUT], f32, isOutput=False)
    xhat_t = nc.declare_dram_parameter("xhat_t", [2 * NOUT, BLK], f32, isOutput=True)

    with tile.TileContext(nc) as tc:
        with (
            tc.tile_pool(name="w", bufs=1) as wp,
            tc.tile_pool(name="adj", bufs=8) as adjp,
            tc.tile_pool(name="usb", bufs=2) as usbp,
            tc.tile_pool(name="out", bufs=2) as outp,
            tc.tile_pool(name="psum", bufs=1, space="PSUM") as psp,
        ):
            # W2 duplicated on both partition halves so each stream's
            # stage-2 matmul finds lhsT/rhs on matching base partitions.
            w2_sb = wp.tile([128, NOUT], f32, tag="w2")
            nc.scalar.dma_start(w2_sb[0:NHID, :], w2[:, :])
            nc.scalar.dma_start(w2_sb[NHID : 2 * NHID, :], w2[:, :])

            z_sb = wp.tile([128, KT, 2 * NHID], bdt, tag="z_sb")
            NCH = 8
            CH = KT // NCH
            for c in range(NCH):
                nc.scalar.dma_start(
                    z_sb[:, c * CH : (c + 1) * CH, :],
                    z_d[c * CH * 128 : (c + 1) * CH * 128, :].rearrange(
                        "(o p) h -> p o h", p=128
                    ),
                )

            psU = [
                psp.tile([128, 512], f32, tag=f"psU{ic}", name=f"psU{ic}")
                for ic in range(IC)
            ]
            for jt in range(KT):
                at = adjp.tile([128, BLK], bdt, tag="adj")
                eng = nc.sync if jt % 2 == 0 else nc.scalar
                eng.dma_start(at[:], adjt[jt * 128 : (jt + 1) * 128, :])
                for ic in range(IC):
                    nc.tensor.matmul(
                        psU[ic][:],
                        z_sb[:, jt, :],
                        at[:, ic * 512 : (ic + 1) * 512],
                        start=(jt == 0),
                        stop=(jt == KT - 1),
                    )

            # xhat.T chunk = W2.T @ U.T chunk per stream (fp32, tiny)
            for ic in range(IC):
                u_sb = usbp.tile([128, 512], f32, tag="u")
                nc.vector.tensor_copy(out=u_sb[:], in_=psU[ic][:])
                for s in range(2):
                    psx = psp.tile(
                        [128, 512],
                        f32,
                        tag=f"psU{ic}" if s == 0 else "psx1",
                        name=f"psx{ic}_{s}",
                    )
                    nc.tensor.matmul(
                        psx[:],
                        w2_sb[s * NHID : (s + 1) * NHID, :],
                        u_sb[s * NHID : (s + 1) * NHID, :],
                        start=True,
                        stop=True,
                    )
                    xo = outp.tile([NOUT, 512], f32, tag="xho")
                    nc.vector.tensor_copy(out=xo[:], in_=psx[:NOUT, :])
                    nc.scalar.dma_start(
                        xhat_t[s * NOUT : (s + 1) * NOUT, ic * 512 : (ic + 1) * 512],
                        xo[:],
                    )
    nc.compile()
    return nc


def _prep(feat, feat_a, fadj, W1, W2):
    feat = np.ascontiguousarray(feat, dtype=np.float32)
    feat_a = np.ascontiguousarray(feat_a, dtype=np.float32)
    fadj = np.ascontiguousarray(fadj, dtype=np.float32)
    W1 = np.ascontiguousarray(W1, dtype=np.float32)
    W2 = np.ascontiguousarray(W2, dtype=np.float32)

    adjt = big_cast(fadj.T)  # [N, N] = (j, i)
    adj_blocks = [
        np.ascontiguousarray(adjt[:, c * BLK : (c + 1) * BLK]) for c in range(NCORES)
    ]
    return adj_blocks, big_cast(feat), big_cast(feat_a), W1, W2


def _run(inputs, trace=False):
    adj_blocks, xo, xa, W1, W2 = _prep(
        inputs["feat"], inputs["feat_a"], inputs["fadj"], inputs["W1"], inputs["W2"]
    )
    core_ids = list(range(NCORES))

    if ("l1", BIG) not in _cache:
        _cache[("l1", BIG)] = build_layer1(big_dt())
    in_maps1 = [
        {"adjt": adj_blocks[c], "xo": xo, "xa": xa, "w1": W1} for c in range(NCORES)
    ]
    r1 = run_bass_kernel_spmd(_cache[("l1", BIG)], in_maps1, core_ids, trace=trace)
    zt_full = np.concatenate([r1.results[c]["z_t"] for c in range(NCORES)], axis=1)

    z_ori = np.ascontiguousarray(zt_full[0:NHID].T)
    z_aug = np.ascontiguousarray(zt_full[NHID : 2 * NHID].T)
    z_nat = big_cast(np.concatenate([z_ori, z_aug], axis=1))  # [N, 128]

    if ("l2", BIG) not in _cache:
        _cache[("l2", BIG)] = build_layer2(big_dt())
    in_maps2 = [
        {"adjt": adj_blocks[c], "z": z_nat, "w2": W2} for c in range(NCORES)
    ]
    r2 = run_bass_kernel_spmd(_cache[("l2", BIG)], in_maps2, core_ids, trace=trace)
    xhatt = np.concatenate([r2.results[c]["xhat_t"] for c in range(NCORES)], axis=1)

    xhat_ori = np.ascontiguousarray(xhatt[0:NOUT].T)
    xhat_aug = np.ascontiguousarray(xhatt[NOUT : 2 * NOUT].T)
    times = (r1.exec_time_ns, r2.exec_time_ns)
    return (z_ori, z_aug, xhat_ori, xhat_aug), times


def kernel(**inputs):
    outputs, _ = _run(inputs, trace=False)
    return outputs
